# revision 16
# baseline (speedup 1.0000x reference)
"""Trainium2 Bass kernel for DifferentiableDLT.

Strategy (pure batch data-parallelism, 8 cores x 64 batches):
  - The 22x22 sample grid is input-independent: bilinear corner indices and
    weights are host constants. Only 44 image rows and 22 16-float column
    chunks (64B-aligned) of each image are touched; those are gathered by DMA
    into packed SBUF row tiles (~12 MB/core instead of 1 GB).
  - flow (64,2,H,W) is viewed as (128,H,W): partition p = batch*2 + channel
    (single HBM stride), so one 3-dim-AP DMA per (row, chunk-series).
  - Bilinear sampling = strided SBUF reads x constant weight vectors (DVE).
  - softmax(10*mask), Hartley normalization of dst points on-chip; src-side
    normalization and T_src are host constants.
  - M (9x9) = A^T A has block structure: 24 weighted reductions
    R[j,k] = sum_p g_j[p]*c_k[p] over 484 points (fused mult+accum on DVE).
  - Smallest eigenvector via LDL^T factorization + 2 inverse-iteration
    solves (vectorized over 64 batches on partitions; matrix in free dim).
  - H = T_dst_inv @ h @ T_src, then H / H22 (h is left unnormalized: the
    reference's H / max(H22, 1e-8) is scale- and sign-invariant whenever the
    normalized |H22| > 1e-8, and LAPACK's sign choice gives H22 > 0 with
    |H22| >= 0.57 across this data distribution, so plain division by the
    signed H22 reproduces it exactly).
"""

import math
import sys

import numpy as np

for _p in ("/opt/trn_rl_repo", "/root/.axon_site/_ro/trn_rl_repo"):
    if _p not in sys.path:
        sys.path.append(_p)

f32 = np.float32

NCORES = 8
BPC = 64          # batches per core
HF, WF = 315, 560
N22 = 22
NPTS = 484
NR = 44           # packed row slots (y0,y0+1 pairs, yi-major)
NCH = 22          # 8-float chunks per row (uniform stride 24)
MROWF = NCH * 8   # 176 floats per packed mask row
HW = HF * WF

# constant-block layout (free-dim offsets into CB)
OFF_W = 0                      # w00,w10,w01,w11: 4*484
OFF_C6 = 4 * NPTS              # c_k = xx,xy,x,yy,y,1 at src_norm: 6*484
OFF_B0 = 10 * NPTS             # inverse-iteration init: 9
OFF_TS = 10 * NPTS + 9         # T_src row-major: 9
CF = 10 * NPTS + 18
CFI = NPTS + 1                 # per-channel: grid*scale (484), scale (1)


def _geometry():
    mh, mw = int(HF * 0.05), int(WF * 0.05)
    ys = np.linspace(mh, HF - mh - 1, N22, dtype=f32)
    xs = np.linspace(mw, WF - mw - 1, N22, dtype=f32)
    y0 = np.floor(ys).astype(np.int64)
    x0 = np.floor(xs).astype(np.int64)
    rows = np.stack([y0, y0 + 1], 1).reshape(-1)
    cstart = (x0 // 8) * 8
    coff = x0 - cstart
    # chunk structure the DMA/sampling APs hardcode:
    # 8-float chunks at uniform stride 24 from col 24; x0 at +4 (xi=0) / +3
    assert np.all(np.diff(cstart) == 24) and cstart[0] == 24
    assert coff[0] == 4 and np.all(coff[1:] == 3)
    # flow spans: x0-24 = 4 | 27+24k for xi>=1; x1 max fits in 512
    assert int(x0[1]) - 24 == 27 and int(x0[21]) + 1 - 24 < 512
    wy = (ys - y0).astype(f32)
    wx = (xs - x0).astype(f32)
    return ys, xs, rows, wy, wx


def _constants(img_h, img_w):
    ys, xs, rows, wy, wx = _geometry()
    WY0 = np.repeat(1 - wy, N22).astype(f32)
    WY1 = np.repeat(wy, N22).astype(f32)
    WX0 = np.tile(1 - wx, N22).astype(f32)
    WX1 = np.tile(wx, N22).astype(f32)
    w00 = WX0 * WY0
    w10 = WX1 * WY0
    w01 = WX0 * WY1
    w11 = WX1 * WY1
    sx, sy = f32(img_w / WF), f32(img_h / HF)
    gx = np.tile(xs, N22).astype(f32)
    gy = np.repeat(ys, N22).astype(f32)
    g2x = (gx * sx).astype(f32)
    g2y = (gy * sy).astype(f32)
    # src Hartley normalization (input-independent), f32 chain like reference
    msx = g2x.mean(dtype=f32).astype(f32)
    msy = g2y.mean(dtype=f32).astype(f32)
    csx = (g2x - msx).astype(f32)
    csy = (g2y - msy).astype(f32)
    rs = np.sqrt(csx * csx + csy * csy, dtype=f32)
    ssc = f32(max(rs.mean(dtype=f32) / f32(math.sqrt(2)), 1e-8))
    si = f32(1.0 / ssc)
    snx = (csx * si).astype(f32)
    sny = (csy * si).astype(f32)
    Tsrc = np.array([si, 0, -msx * si, 0, si, -msy * si, 0, 0, 1], dtype=f32)
    b0 = (np.array([1, 0, 0, 0, 1, 0, 0, 0, 1], f32) / f32(math.sqrt(3))).astype(f32)
    cb = np.zeros((1, CF), f32)
    cb[0, 0:NPTS] = w00
    cb[0, NPTS:2 * NPTS] = w10
    cb[0, 2 * NPTS:3 * NPTS] = w01
    cb[0, 3 * NPTS:4 * NPTS] = w11
    c6 = [snx * snx, snx * sny, snx, sny * sny, sny, np.ones(NPTS, f32)]
    for k in range(6):
        cb[0, OFF_C6 + k * NPTS:OFF_C6 + (k + 1) * NPTS] = c6[k].astype(f32)
    cb[0, OFF_B0:OFF_B0 + 9] = b0
    cb[0, OFF_TS:OFF_TS + 9] = Tsrc
    cbi = np.zeros((2, CFI), f32)
    cbi[0, 0:NPTS] = g2x
    cbi[0, NPTS] = sx
    cbi[1, 0:NPTS] = g2y
    cbi[1, NPTS] = sy
    return cb, cbi, rows


def _split_multi_waits(nc, maxw=1):
    """This walrus build rejects >1 sem-wait per instruction: hoist extras
    onto injected single-wait NoOps just before the carrier."""
    import concourse.mybir as mybir

    cnt = 0
    for fn in nc.m.functions:
        for bb in fn.blocks:
            out = []
            for ins in bb.instructions:
                si = ins.sync_info
                if si is not None and si.on_wait is not None and len(si.on_wait) > maxw:
                    waits = list(si.on_wait)
                    for w in waits[maxw:]:
                        cnt += 1
                        out.append(mybir.InstNoOp(
                            name=f"I-wsplit-{cnt}",
                            engine=ins.engine,
                            sync_info=mybir.SyncInfo(on_wait=[w], on_update=[]),
                        ))
                    si.on_wait = waits[:maxw]
                out.append(ins)
            bb.instructions = out
    return cnt


def _fap(base, off, dims):
    """Free-dim AP on an SBUF tile view: keep partition dim, custom free dims."""
    import concourse.bass as bass
    return bass.AP(tensor=base.tensor, offset=base.offset + off,
                   ap=[list(base.ap[0])] + [list(d) for d in dims])


def _build_program(rows, debug=False):
    import concourse.bass as bass
    import concourse.tile as tile
    from concourse import mybir

    dt = mybir.dt.float32
    op = mybir.AluOpType
    act = mybir.ActivationFunctionType

    nc = bass.Bass(trn_type="TRN2")
    flow_d = nc.dram_tensor("flow", [BPC, 2, HF, WF], dt, kind="ExternalInput")
    mask_d = nc.dram_tensor("mask", [BPC, 1, HF, WF], dt, kind="ExternalInput")
    cb_d = nc.dram_tensor("cb", [1, CF], dt, kind="ExternalInput")
    cbi_d = nc.dram_tensor("cbi", [2, CFI], dt, kind="ExternalInput")
    hout_d = nc.dram_tensor("hout", [BPC, 9], dt, kind="ExternalOutput")

    taps = []

    def tap(name, t, view=None):
        if not debug:
            return
        ap = t[:] if view is None else view
        pd = ap.shape[0]
        fs = 1
        for s in ap.shape[1:]:
            fs *= s
        d = nc.dram_tensor(f"dbg_{name}", [pd, fs], dt, kind="ExternalOutput")
        nc.sync.dma_start(out=d[:], in_=ap)
        taps.append(name)

    # flow span geometry: FR row-slot s holds image row rows[s], 512 floats
    # from col 24; slot(yi, dy) = 2*yi + dy.
    SPAN = 512
    X0 = 24
    y0 = rows[0::2]  # 22 y0s, yi-major

    with tile.TileContext(nc) as tc:
        with tc.tile_pool(name="p", bufs=1) as pool:
            CB = pool.tile([128, CF], dt)
            CBI = pool.tile([128, CFI], dt)
            FR = pool.tile([128, NR, SPAN], dt)

            nc.sync.dma_start(out=CB[:], in_=bass.AP(
                tensor=cb_d[:].tensor, offset=0, ap=[[0, 128], [1, CF]]))
            nc.sync.dma_start(out=CBI[:], in_=bass.AP(
                tensor=cbi_d[:].tensor, offset=0, ap=[[0, 64], [CFI, 2], [1, CFI]]))

            # --- flow rows: 5 span DMAs (row series with uniform stride) ---
            # series: (slot0, slot_stride, count, img_row0, img_row_stride)
            assert np.all(np.diff(y0[0::2]) == 27) and np.all(np.diff(y0[1::2][:10]) == 27)
            series = [
                (0, 4, 11, int(y0[0]), 27),        # yi even, dy=0
                (1, 4, 11, int(y0[0]) + 1, 27),    # yi even, dy=1
                (2, 4, 10, int(y0[1]), 27),        # yi odd (first 10), dy=0
                (3, 4, 10, int(y0[1]) + 1, 27),    # yi odd, dy=1
            ]
            def flow_span(eng, s0, ss, cnt, ir0, irs):
                srcap = bass.AP(tensor=flow_d[:].tensor,
                                offset=ir0 * WF + X0,
                                ap=[[HW, 128], [irs * WF, cnt], [1, SPAN]])
                eng.dma_start(
                    out=_fap(FR[:], s0 * SPAN, [[ss * SPAN, cnt], [1, SPAN]]),
                    in_=srcap)

            for (s0, ss, cnt, ir0, irs) in series[:2]:   # A, A1 -> SP
                flow_span(nc.sync, s0, ss, cnt, ir0, irs)

            # --- mask rows: two phases of 22 row-slots (yi 0-10 / 11-21),
            # span DMAs (few large contiguous segments), shared buffer ---
            # phase A series: (slot0, slot_stride, count, img_row0, img_row_stride)
            mseriesA = [
                (0, 4, 6, int(y0[0]), 27),       # yi even 0..10, dy=0
                (1, 4, 6, int(y0[0]) + 1, 27),
                (2, 4, 5, int(y0[1]), 27),       # yi odd 1..9, dy=0
                (3, 4, 5, int(y0[1]) + 1, 27),
            ]
            mseriesB = [
                (0, 4, 5, int(y0[11]), 27),      # yi odd 11..19 -> local 0,2,..
                (1, 4, 5, int(y0[11]) + 1, 27),
                (2, 4, 5, int(y0[12]), 27),      # yi even 12..20 -> local 1,3,..
                (3, 4, 5, int(y0[12]) + 1, 27),
                (20, 1, 2, int(y0[21]), 1),      # yi 21 pair -> slots 20,21
            ]

            def mask_dma(mt, series):
                for (s0, ss, cnt, ir0, irs) in series:
                    srcap = bass.AP(tensor=mask_d[:].tensor,
                                    offset=ir0 * WF + X0,
                                    ap=[[HW, 64], [irs * WF, cnt], [1, SPAN]])
                    nc.scalar.dma_start(
                        out=_fap(mt[:], s0 * SPAN, [[ss * SPAN, cnt], [1, SPAN]]),
                        in_=srcap)

            # --- bilinear sampling ---
            # flow pieces (span layout): x0-X0 = 4 (xi=0) | 27+24k (xi>=1)
            fpieces = [
                (4,  [[2 * SPAN, 22]],            0, [[N22, 22]]),
                (27, [[2 * SPAN, 22], [24, 21]],  1, [[N22, 22], [1, 21]]),
            ]
            terms = [(0, 0, 0), (0, 1, 1), (1, 0, 2), (1, 1, 3)]  # dy, dx, w-idx

            def sample(rowtile, rowf, pieces, out_t, tmp_t, nyi=22, yi0=0):
                # writes out_t free range [yi0*22, (yi0+nyi)*22)
                accs = []
                po = yi0 * N22
                for (dy, dx, wi) in terms:
                    t = tmp_t[wi]
                    for (ib, idims, ob, odims) in pieces:
                        idims = [[idims[0][0], nyi]] + idims[1:]
                        odims = [[odims[0][0], nyi]] + odims[1:]
                        nc.vector.tensor_mul(
                            _fap(t[:], ob, odims),
                            _fap(rowtile[:], dy * rowf + ib + dx, idims),
                            _fap(CB[0:t[:].shape[0]],
                                 OFF_W + wi * NPTS + po + ob, odims))
                    accs.append(t)
                w = nyi * N22
                dst = out_t[:, po:po + w]
                nc.vector.tensor_add(dst, accs[0][:, 0:w], accs[1][:, 0:w])
                nc.vector.tensor_add(dst, dst, accs[2][:, 0:w])
                nc.vector.tensor_add(dst, dst, accs[3][:, 0:w])

            SMM = pool.tile([64, NPTS], dt)
            for phase, mseries in enumerate((mseriesA, mseriesB)):
                MT = pool.tile([64, 22, SPAN], dt, tag="mrows")
                mask_dma(MT, mseries)
                tmps = [pool.tile([64, 11 * N22], dt, tag=f"t{i}",
                                  name=f"mtmp{phase}_{i}")
                        for i in range(4)]
                sample(MT, SPAN, fpieces, SMM, tmps, nyi=11, yi0=11 * phase)
                if phase == 0:
                    for (s0, ss, cnt, ir0, irs) in series[2:]:  # B, B1 -> ACT
                        flow_span(nc.scalar, s0, ss, cnt, ir0, irs)
                    flow_span(nc.scalar, 42, 1, 2, int(y0[21]), 1)  # C
            tap('SMM', SMM)

            # --- softmax weights (needs only mask) ---
            MX = pool.tile([64, 1], dt)
            NM10 = pool.tile([64, 1], dt)
            EW = pool.tile([64, NPTS], dt)
            SE = pool.tile([64, 1], dt)
            RS = pool.tile([64, 1], dt)
            nc.vector.reduce_max(MX[:], SMM[:], axis=mybir.AxisListType.X)
            nc.vector.tensor_scalar_mul(NM10[:], MX[:], -10.0)
            nc.scalar.activation(EW[:], SMM[:], act.Exp,
                                 bias=NM10[:], scale=10.0, accum_out=SE[:])
            nc.vector.reciprocal(RS[:], SE[:])
            nc.vector.tensor_scalar_mul(EW[:], EW[:], RS[:])
            tap('W', EW)
            nc.vector.tensor_mul(EW[:], EW[:], EW[:])
            W2 = EW

            # --- flow sampling (after flow spans land; reuses t0-t3 slots) ---
            # yi-even group reads slots 4m/4m+1 (series A/A1 only);
            # yi-odd group reads slots 4m+2/4m+3 (series B/B1/C).
            SMF = pool.tile([128, NPTS], dt)
            for par in range(2):
                tmps = [pool.tile([128, 11 * N22], dt, tag=f"t{i}",
                                  name=f"ftmp{par}_{i}")
                        for i in range(4)]
                accs = []
                n_yi = 11
                for (dy, dx, wi) in terms:
                    t = tmps[wi]
                    for (ib, idims, ob, odims) in fpieces:
                        idims = [[4 * SPAN, n_yi]] + idims[1:]
                        odims = [[odims[0][0], n_yi]] + odims[1:]
                        nc.vector.tensor_mul(
                            _fap(t[:], ob, odims),
                            _fap(FR[:], (2 * par + dy) * SPAN + ib + dx, idims),
                            bass.AP(tensor=CB[:].tensor,
                                    offset=CB[:].offset + OFF_W + wi * NPTS
                                    + par * N22 + ob,
                                    ap=[list(CB[:].ap[0]),
                                        [2 * N22, n_yi]] + odims[1:]))
                    accs.append(t)
                # accumulate into SMF strided columns (yi = 2m+par)
                dsta = bass.AP(tensor=SMF[:].tensor,
                               offset=SMF[:].offset + par * N22,
                               ap=[list(SMF[:].ap[0]), [2 * N22, n_yi], [1, N22]])
                nc.vector.tensor_add(dsta, _fap(accs[0][:], 0, [[N22, n_yi], [1, N22]]),
                                     _fap(accs[1][:], 0, [[N22, n_yi], [1, N22]]))
                nc.vector.tensor_add(dsta, dsta,
                                     _fap(accs[2][:], 0, [[N22, n_yi], [1, N22]]))
                nc.vector.tensor_add(dsta, dsta,
                                     _fap(accs[3][:], 0, [[N22, n_yi], [1, N22]]))
            tap('SMF', SMF)

            # --- dst points + Hartley (DP/CD in place on SMF) ---
            MN = pool.tile([128, 1], dt)
            nc.vector.scalar_tensor_tensor(
                SMF[:], SMF[:], CBI[:, NPTS:NPTS + 1], CBI[:, 0:NPTS],
                op.mult, op.add)
            tap('DP', SMF)
            nc.vector.reduce_sum(MN[:], SMF[:], axis=mybir.AxisListType.X)
            nc.vector.tensor_scalar_mul(MN[:], MN[:], 1.0 / NPTS)
            nc.vector.tensor_scalar(SMF[:], SMF[:], MN[:], None, op.subtract)

            CDX = pool.tile([64, NPTS], dt)
            CDY = pool.tile([64, NPTS], dt)
            MNP = pool.tile([64, 2], dt)
            cdap = SMF[:]
            pstep = cdap.ap[0][0]
            nc.sync.dma_start(out=CDX[:], in_=bass.AP(
                tensor=cdap.tensor, offset=cdap.offset, ap=[[2 * pstep, 64], [1, NPTS]]))
            nc.sync.dma_start(out=CDY[:], in_=bass.AP(
                tensor=cdap.tensor, offset=cdap.offset + pstep,
                ap=[[2 * pstep, 64], [1, NPTS]]))
            mnap = MN[:]
            nc.sync.dma_start(out=MNP[:], in_=bass.AP(
                tensor=mnap.tensor, offset=mnap.offset, ap=[[mnap.ap[0][0], 128], [1, 1]]))
            tap('CDX', CDX)
            tap('CDY', CDY)
            tap('MNP', MNP)

            R2 = pool.tile([64, NPTS], dt)
            TG = pool.tile([64, NPTS], dt)
            RT = pool.tile([64, NPTS], dt)
            SR = pool.tile([64, 1], dt)
            SC = pool.tile([64, 1], dt)
            IV = pool.tile([64, 1], dt)
            IV2 = pool.tile([64, 1], dt)
            nc.vector.tensor_mul(R2[:], CDX[:], CDX[:])
            nc.vector.tensor_mul(TG[:], CDY[:], CDY[:])
            nc.vector.tensor_add(R2[:], R2[:], TG[:])
            nc.scalar.activation(RT[:], R2[:], act.Sqrt, accum_out=SR[:])
            nc.vector.tensor_scalar(SC[:], SR[:], 1.0 / (NPTS * math.sqrt(2.0)),
                                    1e-8, op.mult, op.max)
            nc.vector.reciprocal(IV[:], SC[:])
            nc.vector.tensor_mul(IV2[:], IV[:], IV[:])
            tap('R2', R2)
            tap('SC', SC)
            tap('IV', IV)
            # normalized dst in place (CDX/CDY -> DNX/DNY)
            nc.vector.tensor_scalar_mul(CDX[:], CDX[:], IV[:])
            nc.vector.tensor_scalar_mul(CDY[:], CDY[:], IV[:])

            # --- g vectors paired on 128 partitions: GU0 = [W2; W2*DNX],
            # GU1 = [W2*DNY; W2*r2n] ---
            GU0 = pool.tile([128, NPTS], dt, tag="t0")
            GU1 = pool.tile([128, NPTS], dt, tag="t1")
            nc.vector.tensor_copy(GU0[0:64, :], W2[:])
            nc.vector.tensor_mul(GU0[64:128, :], W2[:], CDX[:])
            nc.vector.tensor_mul(GU1[0:64, :], W2[:], CDY[:])
            nc.vector.tensor_scalar_mul(TG[:], R2[:], IV2[:])
            nc.vector.tensor_mul(GU1[64:128, :], TG[:], W2[:])

            JK = pool.tile([128, NPTS], dt, tag="t2")
            RD2 = pool.tile([128, 12], dt)
            RD = pool.tile([64, 24], dt)
            for gj, GU in enumerate([GU0, GU1]):
                for k in range(6):
                    nc.vector.scalar_tensor_tensor(
                        JK[:], GU[:], 1.0,
                        CB[:, OFF_C6 + k * NPTS:OFF_C6 + (k + 1) * NPTS],
                        op.bypass, op.mult,
                        accum_out=RD2[:, gj * 6 + k:gj * 6 + k + 1])
            # RD2 partition halves -> RD (64, 24): j order 0,1,2,3
            nc.vector.tensor_copy(_fap(RD[:], 0, [[12, 2], [1, 6]]),
                                  _fap(RD2[0:64], 0, [[6, 2], [1, 6]]))
            nc.vector.tensor_copy(_fap(RD[:], 6, [[12, 2], [1, 6]]),
                                  _fap(RD2[64:128], 0, [[6, 2], [1, 6]]))
            tap('RD', RD)

            # --- assemble M (64, 81) ---
            MM = pool.tile([64, 81], dt)
            EB = pool.tile([64, 4, 9], dt)
            nc.vector.memset(MM[:], 0.0)
            # negate j=1,2 (the -B1/-B2 blocks), then 4 strided copies build
            # all four 9-entry blocks at once: E[j] = RD[j][0,1,2,1,3,4,2,4,5]
            nc.vector.tensor_scalar_mul(RD[:, 6:18], RD[:, 6:18], -1.0)
            nc.vector.tensor_copy(_fap(EB[:], 0, [[9, 4], [1, 3]]),
                                  _fap(RD[:], 0, [[6, 4], [1, 3]]))
            nc.vector.tensor_copy(_fap(EB[:], 3, [[9, 4], [3, 2]]),
                                  _fap(RD[:], 1, [[6, 4], [1, 2]]))
            nc.vector.tensor_copy(_fap(EB[:], 4, [[9, 4], [1, 2]]),
                                  _fap(RD[:], 3, [[6, 4], [1, 2]]))
            nc.vector.tensor_copy(_fap(EB[:], 7, [[9, 4], [1, 2]]),
                                  _fap(RD[:], 4, [[6, 4], [1, 2]]))
            for j, boff in ((0, 0), (0, 30), (3, 60),
                            (1, 6), (1, 54), (2, 33), (2, 57)):
                nc.vector.tensor_copy(
                    _fap(MM[:], boff, [[9, 3], [1, 3]]),
                    _fap(EB[:], j * 9, [[3, 3], [1, 3]]))
            nc.vector.tensor_scalar_add(
                _fap(MM[:], 0, [[10, 9]]), _fap(MM[:], 0, [[10, 9]]), 1e-6)
            if debug:
                MMC = pool.tile([64, 81], dt)
                nc.vector.tensor_copy(MMC[:], MM[:])
                tap('MMpost', MMC)

            # --- LDL^T (in place; strictly-lower cols end up holding -L) ---
            ID = pool.tile([64, 9], dt)
            TMPO = pool.tile([64, 64], dt)
            for k in range(9):
                nc.vector.reciprocal(ID[:, k:k + 1], MM[:, 10 * k:10 * k + 1])
                if k < 8:
                    m = 8 - k
                    col = _fap(MM[:], (k + 1) * 9 + k, [[9, m]])
                    nc.vector.tensor_scalar(col, col, ID[:, k:k + 1], -1.0,
                                            op.mult, op.mult)
                    nc.vector.tensor_mul(
                        _fap(TMPO[:], 0, [[m, m], [1, m]]),
                        _fap(MM[:], (k + 1) * 9 + k, [[9, m], [0, m]]),
                        _fap(MM[:], 9 * k + k + 1, [[0, m], [1, m]]))
                    sub = _fap(MM[:], (k + 1) * 10, [[9, m], [1, m]])
                    nc.vector.tensor_add(sub, sub,
                                         _fap(TMPO[:], 0, [[m, m], [1, m]]))
            tap('ID', ID)

            # --- inverse iteration (2 solves) ---
            Z = pool.tile([64, 9], dt)
            Y = pool.tile([64, 9], dt)
            XN = pool.tile([64, 9], dt)
            TB1 = pool.tile([64, 1], dt)
            for it in range(2):
                if it == 0:
                    nc.vector.tensor_copy(Z[:], CB[0:64, OFF_B0:OFF_B0 + 9])
                    ZT = Z
                else:
                    ZT = XN  # iter-2 forward solve runs in place on XN
                for k in range(8):
                    m = 8 - k
                    nc.vector.scalar_tensor_tensor(
                        ZT[:, k + 1:9],
                        _fap(MM[:], (k + 1) * 9 + k, [[9, m]]),
                        ZT[:, k:k + 1], ZT[:, k + 1:9], op.mult, op.add)
                nc.vector.tensor_mul(Y[:], ZT[:], ID[:])
                nc.vector.tensor_copy(XN[:, 8:9], Y[:, 8:9])
                for k in range(7, -1, -1):
                    m = 8 - k
                    nc.vector.scalar_tensor_tensor(
                        TMPO[:, 0:m],
                        _fap(MM[:], (k + 1) * 9 + k, [[9, m]]),
                        1.0, XN[:, k + 1:9], op.bypass, op.mult,
                        accum_out=TB1[:])
                    nc.vector.tensor_add(XN[:, k:k + 1], Y[:, k:k + 1], TB1[:])
            tap('XN', XN)

            # --- H = T_dst_inv @ (h @ T_src), sign fix, scale ---
            H1 = pool.tile([64, 9], dt)
            TT = pool.tile([64, 9], dt)
            for k in range(3):
                hcol = _fap(XN[:], k, [[3, 3], [0, 3]])
                trow = _fap(CB[0:64], OFF_TS + 3 * k, [[0, 3], [1, 3]])
                if k == 0:
                    nc.vector.tensor_mul(H1[:], hcol, trow)
                else:
                    nc.vector.tensor_mul(TT[:], hcol, trow)
                    nc.vector.tensor_add(H1[:], H1[:], TT[:])
            tap('H1', H1)
            HO = pool.tile([64, 9], dt)
            TB3 = pool.tile([64, 3], dt)
            nc.vector.tensor_scalar_mul(TB3[:], H1[:, 6:9], MNP[:, 0:1])
            nc.vector.scalar_tensor_tensor(HO[:, 0:3], H1[:, 0:3], SC[:], TB3[:],
                                           op.mult, op.add)
            nc.vector.tensor_scalar_mul(TB3[:], H1[:, 6:9], MNP[:, 1:2])
            nc.vector.scalar_tensor_tensor(HO[:, 3:6], H1[:, 3:6], SC[:], TB3[:],
                                           op.mult, op.add)
            nc.vector.tensor_copy(HO[:, 6:9], H1[:, 6:9])

            FF = pool.tile([64, 1], dt)
            nc.vector.reciprocal(FF[:], HO[:, 8:9])
            nc.vector.tensor_scalar_mul(HO[:], HO[:], FF[:])

            nc.sync.dma_start(out=hout_d[:], in_=HO[:])

    _split_multi_waits(nc)
    nc._dbg_taps = taps
    return nc


_PROG = {}


def _get_prog(img_h, img_w):
    key = (int(img_h), int(img_w))
    if key not in _PROG:
        cb, cbi, rows = _constants(*key)
        nc = _build_program(rows)
        _PROG[key] = (nc, cb, cbi)
    return _PROG[key]


def _make_in_maps(flow, mask, cb, cbi):
    in_maps = []
    for c in range(NCORES):
        sl = slice(c * BPC, (c + 1) * BPC)
        in_maps.append({
            "flow": np.ascontiguousarray(flow[sl]),
            "mask": np.ascontiguousarray(mask[sl]),
            "cb": cb,
            "cbi": cbi,
        })
    return in_maps


def run_spmd(flow, mask, img_h, img_w, **kw):
    """Compile (cached) + run on 8 cores. Returns BassKernelResults."""
    from concourse.bass_utils import run_bass_kernel_spmd
    nc, cb, cbi = _get_prog(img_h, img_w)
    in_maps = _make_in_maps(flow, mask, cb, cbi)
    return run_bass_kernel_spmd(nc, in_maps, core_ids=list(range(NCORES)), **kw)


def kernel(flow, mask, img_h, img_w):
    flow = np.ascontiguousarray(np.asarray(flow), dtype=f32)
    mask = np.ascontiguousarray(np.asarray(mask), dtype=f32)
    res = run_spmd(flow, mask, int(img_h), int(img_w))
    out = np.concatenate([r["hout"] for r in res.results], axis=0)
    return out.reshape(flow.shape[0], 3, 3).astype(f32)


# revision 17
# speedup vs baseline: 1.0662x; 1.0662x over previous
"""Trainium2 Bass kernel for DifferentiableDLT.

Strategy (pure batch data-parallelism, 8 cores x 64 batches):
  - The 22x22 sample grid is input-independent: bilinear corner indices and
    weights are host constants. Only 44 image rows and 22 16-float column
    chunks (64B-aligned) of each image are touched; those are gathered by DMA
    into packed SBUF row tiles (~12 MB/core instead of 1 GB).
  - flow (64,2,H,W) is viewed as (128,H,W): partition p = batch*2 + channel
    (single HBM stride), so one 3-dim-AP DMA per (row, chunk-series).
  - Bilinear sampling = strided SBUF reads x constant weight vectors (DVE).
  - softmax(10*mask), Hartley normalization of dst points on-chip; src-side
    normalization and T_src are host constants.
  - M (9x9) = A^T A has block structure: 24 weighted reductions
    R[j,k] = sum_p g_j[p]*c_k[p] over 484 points (fused mult+accum on DVE).
  - Smallest eigenvector via LDL^T factorization + 2 inverse-iteration
    solves (vectorized over 64 batches on partitions; matrix in free dim).
  - H = T_dst_inv @ h @ T_src, then H / H22 (h is left unnormalized: the
    reference's H / max(H22, 1e-8) is scale- and sign-invariant whenever the
    normalized |H22| > 1e-8, and LAPACK's sign choice gives H22 > 0 with
    |H22| >= 0.57 across this data distribution, so plain division by the
    signed H22 reproduces it exactly).
"""

import math
import sys

import numpy as np

for _p in ("/opt/trn_rl_repo", "/root/.axon_site/_ro/trn_rl_repo"):
    if _p not in sys.path:
        sys.path.append(_p)

f32 = np.float32

NCORES = 8
BPC = 64          # batches per core
HF, WF = 315, 560
N22 = 22
NPTS = 484
NR = 44           # packed row slots (y0,y0+1 pairs, yi-major)
NCH = 22          # 8-float chunks per row (uniform stride 24)
MROWF = NCH * 8   # 176 floats per packed mask row
HW = HF * WF

# constant-block layout (free-dim offsets into CB)
OFF_W = 0                      # w00,w10,w01,w11: 4*484
OFF_C6 = 4 * NPTS              # c_k = xx,xy,x,yy,y,1 at src_norm: 6*484
OFF_B0 = 10 * NPTS             # inverse-iteration init: 9
OFF_TS = 10 * NPTS + 9         # T_src row-major: 9
CF = 10 * NPTS + 18
CFI = NPTS + 1                 # per-channel: grid*scale (484), scale (1)


def _geometry():
    mh, mw = int(HF * 0.05), int(WF * 0.05)
    ys = np.linspace(mh, HF - mh - 1, N22, dtype=f32)
    xs = np.linspace(mw, WF - mw - 1, N22, dtype=f32)
    y0 = np.floor(ys).astype(np.int64)
    x0 = np.floor(xs).astype(np.int64)
    rows = np.stack([y0, y0 + 1], 1).reshape(-1)
    cstart = (x0 // 8) * 8
    coff = x0 - cstart
    # chunk structure the DMA/sampling APs hardcode:
    # 8-float chunks at uniform stride 24 from col 24; x0 at +4 (xi=0) / +3
    assert np.all(np.diff(cstart) == 24) and cstart[0] == 24
    assert coff[0] == 4 and np.all(coff[1:] == 3)
    # flow spans: x0-24 = 4 | 27+24k for xi>=1; x1 max fits in 512
    assert int(x0[1]) - 24 == 27 and int(x0[21]) + 1 - 24 < 512
    wy = (ys - y0).astype(f32)
    wx = (xs - x0).astype(f32)
    return ys, xs, rows, wy, wx


def _constants(img_h, img_w):
    ys, xs, rows, wy, wx = _geometry()
    WY0 = np.repeat(1 - wy, N22).astype(f32)
    WY1 = np.repeat(wy, N22).astype(f32)
    WX0 = np.tile(1 - wx, N22).astype(f32)
    WX1 = np.tile(wx, N22).astype(f32)
    w00 = WX0 * WY0
    w10 = WX1 * WY0
    w01 = WX0 * WY1
    w11 = WX1 * WY1
    sx, sy = f32(img_w / WF), f32(img_h / HF)
    gx = np.tile(xs, N22).astype(f32)
    gy = np.repeat(ys, N22).astype(f32)
    g2x = (gx * sx).astype(f32)
    g2y = (gy * sy).astype(f32)
    # src Hartley normalization (input-independent), f32 chain like reference
    msx = g2x.mean(dtype=f32).astype(f32)
    msy = g2y.mean(dtype=f32).astype(f32)
    csx = (g2x - msx).astype(f32)
    csy = (g2y - msy).astype(f32)
    rs = np.sqrt(csx * csx + csy * csy, dtype=f32)
    ssc = f32(max(rs.mean(dtype=f32) / f32(math.sqrt(2)), 1e-8))
    si = f32(1.0 / ssc)
    snx = (csx * si).astype(f32)
    sny = (csy * si).astype(f32)
    Tsrc = np.array([si, 0, -msx * si, 0, si, -msy * si, 0, 0, 1], dtype=f32)
    b0 = (np.array([1, 0, 0, 0, 1, 0, 0, 0, 1], f32) / f32(math.sqrt(3))).astype(f32)
    cb = np.zeros((1, CF), f32)
    cb[0, 0:NPTS] = w00
    cb[0, NPTS:2 * NPTS] = w10
    cb[0, 2 * NPTS:3 * NPTS] = w01
    cb[0, 3 * NPTS:4 * NPTS] = w11
    c6 = [snx * snx, snx * sny, snx, sny * sny, sny, np.ones(NPTS, f32)]
    for k in range(6):
        cb[0, OFF_C6 + k * NPTS:OFF_C6 + (k + 1) * NPTS] = c6[k].astype(f32)
    cb[0, OFF_B0:OFF_B0 + 9] = b0
    cb[0, OFF_TS:OFF_TS + 9] = Tsrc
    cbi = np.zeros((2, CFI), f32)
    cbi[0, 0:NPTS] = g2x
    cbi[0, NPTS] = sx
    cbi[1, 0:NPTS] = g2y
    cbi[1, NPTS] = sy
    return cb, cbi, rows


def _split_multi_waits(nc, maxw=1):
    """This walrus build rejects >1 sem-wait per instruction: hoist extras
    onto injected single-wait NoOps just before the carrier."""
    import concourse.mybir as mybir

    cnt = 0
    for fn in nc.m.functions:
        for bb in fn.blocks:
            out = []
            for ins in bb.instructions:
                si = ins.sync_info
                if si is not None and si.on_wait is not None and len(si.on_wait) > maxw:
                    waits = list(si.on_wait)
                    for w in waits[maxw:]:
                        cnt += 1
                        out.append(mybir.InstNoOp(
                            name=f"I-wsplit-{cnt}",
                            engine=ins.engine,
                            sync_info=mybir.SyncInfo(on_wait=[w], on_update=[]),
                        ))
                    si.on_wait = waits[:maxw]
                out.append(ins)
            bb.instructions = out
    return cnt


def _fap(base, off, dims):
    """Free-dim AP on an SBUF tile view: keep partition dim, custom free dims."""
    import concourse.bass as bass
    return bass.AP(tensor=base.tensor, offset=base.offset + off,
                   ap=[list(base.ap[0])] + [list(d) for d in dims])


def _build_program(rows, debug=False):
    import concourse.bass as bass
    import concourse.tile as tile
    from concourse import mybir

    dt = mybir.dt.float32
    op = mybir.AluOpType
    act = mybir.ActivationFunctionType

    nc = bass.Bass(trn_type="TRN2")
    flow_d = nc.dram_tensor("flow", [BPC, 2, HF, WF], dt, kind="ExternalInput")
    mask_d = nc.dram_tensor("mask", [BPC, 1, HF, WF], dt, kind="ExternalInput")
    cb_d = nc.dram_tensor("cb", [1, CF], dt, kind="ExternalInput")
    cbi_d = nc.dram_tensor("cbi", [2, CFI], dt, kind="ExternalInput")
    hout_d = nc.dram_tensor("hout", [BPC, 9], dt, kind="ExternalOutput")

    taps = []

    def tap(name, t, view=None):
        if not debug:
            return
        ap = t[:] if view is None else view
        pd = ap.shape[0]
        fs = 1
        for s in ap.shape[1:]:
            fs *= s
        d = nc.dram_tensor(f"dbg_{name}", [pd, fs], dt, kind="ExternalOutput")
        nc.sync.dma_start(out=d[:], in_=ap)
        taps.append(name)

    # flow span geometry: FR row-slot s holds image row rows[s], 512 floats
    # from col 24; slot(yi, dy) = 2*yi + dy.
    SPAN = 512
    X0 = 24
    y0 = rows[0::2]  # 22 y0s, yi-major

    with tile.TileContext(nc) as tc:
        with tc.tile_pool(name="p", bufs=1) as pool:
            CB = pool.tile([128, CF], dt)
            CBI = pool.tile([128, CFI], dt)
            FR = pool.tile([128, NR, SPAN], dt)

            nc.sync.dma_start(out=CB[:], in_=bass.AP(
                tensor=cb_d[:].tensor, offset=0, ap=[[0, 128], [1, CF]]))
            nc.sync.dma_start(out=CBI[:], in_=bass.AP(
                tensor=cbi_d[:].tensor, offset=0, ap=[[0, 64], [CFI, 2], [1, CFI]]))

            # --- flow rows: 5 span DMAs (row series with uniform stride) ---
            # series: (slot0, slot_stride, count, img_row0, img_row_stride)
            assert np.all(np.diff(y0[0::2]) == 27) and np.all(np.diff(y0[1::2][:10]) == 27)
            series = [
                (0, 4, 11, int(y0[0]), 27),        # yi even, dy=0
                (1, 4, 11, int(y0[0]) + 1, 27),    # yi even, dy=1
                (2, 4, 10, int(y0[1]), 27),        # yi odd (first 10), dy=0
                (3, 4, 10, int(y0[1]) + 1, 27),    # yi odd, dy=1
            ]
            def flow_span(eng, s0, ss, cnt, ir0, irs):
                srcap = bass.AP(tensor=flow_d[:].tensor,
                                offset=ir0 * WF + X0,
                                ap=[[HW, 128], [irs * WF, cnt], [1, SPAN]])
                eng.dma_start(
                    out=_fap(FR[:], s0 * SPAN, [[ss * SPAN, cnt], [1, SPAN]]),
                    in_=srcap)

            for (s0, ss, cnt, ir0, irs) in series:
                flow_span(nc.sync, s0, ss, cnt, ir0, irs)
            flow_span(nc.sync, 42, 1, 2, int(y0[21]), 1)  # last pair (yi=21)

            # --- mask rows: two phases of 22 row-slots (yi 0-10 / 11-21),
            # span DMAs (few large contiguous segments), shared buffer ---
            # phase A series: (slot0, slot_stride, count, img_row0, img_row_stride)
            mseriesA = [
                (0, 4, 6, int(y0[0]), 27),       # yi even 0..10, dy=0
                (1, 4, 6, int(y0[0]) + 1, 27),
                (2, 4, 5, int(y0[1]), 27),       # yi odd 1..9, dy=0
                (3, 4, 5, int(y0[1]) + 1, 27),
            ]
            mseriesB = [
                (0, 4, 5, int(y0[11]), 27),      # yi odd 11..19 -> local 0,2,..
                (1, 4, 5, int(y0[11]) + 1, 27),
                (2, 4, 5, int(y0[12]), 27),      # yi even 12..20 -> local 1,3,..
                (3, 4, 5, int(y0[12]) + 1, 27),
                (20, 1, 2, int(y0[21]), 1),      # yi 21 pair -> slots 20,21
            ]

            def mask_dma(mt, series):
                for (s0, ss, cnt, ir0, irs) in series:
                    srcap = bass.AP(tensor=mask_d[:].tensor,
                                    offset=ir0 * WF + X0,
                                    ap=[[HW, 64], [irs * WF, cnt], [1, SPAN]])
                    nc.scalar.dma_start(
                        out=_fap(mt[:], s0 * SPAN, [[ss * SPAN, cnt], [1, SPAN]]),
                        in_=srcap)

            # --- bilinear sampling ---
            # flow pieces (span layout): x0-X0 = 4 (xi=0) | 27+24k (xi>=1)
            fpieces = [
                (4,  [[2 * SPAN, 22]],            0, [[N22, 22]]),
                (27, [[2 * SPAN, 22], [24, 21]],  1, [[N22, 22], [1, 21]]),
            ]
            terms = [(0, 0, 0), (0, 1, 1), (1, 0, 2), (1, 1, 3)]  # dy, dx, w-idx

            def sample(rowtile, rowf, pieces, out_t, tmp_t, nyi=22, yi0=0):
                # writes out_t free range [yi0*22, (yi0+nyi)*22)
                accs = []
                po = yi0 * N22
                for (dy, dx, wi) in terms:
                    t = tmp_t[wi]
                    for (ib, idims, ob, odims) in pieces:
                        idims = [[idims[0][0], nyi]] + idims[1:]
                        odims = [[odims[0][0], nyi]] + odims[1:]
                        nc.vector.tensor_mul(
                            _fap(t[:], ob, odims),
                            _fap(rowtile[:], dy * rowf + ib + dx, idims),
                            _fap(CB[0:t[:].shape[0]],
                                 OFF_W + wi * NPTS + po + ob, odims))
                    accs.append(t)
                w = nyi * N22
                dst = out_t[:, po:po + w]
                nc.vector.tensor_add(dst, accs[0][:, 0:w], accs[1][:, 0:w])
                nc.vector.tensor_add(dst, dst, accs[2][:, 0:w])
                nc.vector.tensor_add(dst, dst, accs[3][:, 0:w])

            SMM = pool.tile([64, NPTS], dt)
            for phase, mseries in enumerate((mseriesA, mseriesB)):
                MT = pool.tile([64, 22, SPAN], dt, tag="mrows")
                mask_dma(MT, mseries)
                tmps = [pool.tile([64, 11 * N22], dt, tag=f"t{i}",
                                  name=f"mtmp{phase}_{i}")
                        for i in range(4)]
                sample(MT, SPAN, fpieces, SMM, tmps, nyi=11, yi0=11 * phase)
            tap('SMM', SMM)

            # --- softmax weights (needs only mask) ---
            MX = pool.tile([64, 1], dt)
            NM10 = pool.tile([64, 1], dt)
            EW = pool.tile([64, NPTS], dt)
            SE = pool.tile([64, 1], dt)
            RS = pool.tile([64, 1], dt)
            nc.vector.reduce_max(MX[:], SMM[:], axis=mybir.AxisListType.X)
            nc.vector.tensor_scalar_mul(NM10[:], MX[:], -10.0)
            nc.scalar.activation(EW[:], SMM[:], act.Exp,
                                 bias=NM10[:], scale=10.0, accum_out=SE[:])
            nc.vector.reciprocal(RS[:], SE[:])
            nc.vector.tensor_scalar_mul(EW[:], EW[:], RS[:])
            tap('W', EW)
            nc.vector.tensor_mul(EW[:], EW[:], EW[:])
            W2 = EW

            # --- flow sampling (after flow spans land; reuses t0-t3 slots) ---
            # yi-even group reads slots 4m/4m+1 (series A/A1 only);
            # yi-odd group reads slots 4m+2/4m+3 (series B/B1/C).
            SMF = pool.tile([128, NPTS], dt)
            for par in range(2):
                tmps = [pool.tile([128, 11 * N22], dt, tag=f"t{i}",
                                  name=f"ftmp{par}_{i}")
                        for i in range(4)]
                accs = []
                n_yi = 11
                for (dy, dx, wi) in terms:
                    t = tmps[wi]
                    for (ib, idims, ob, odims) in fpieces:
                        idims = [[4 * SPAN, n_yi]] + idims[1:]
                        odims = [[odims[0][0], n_yi]] + odims[1:]
                        nc.vector.tensor_mul(
                            _fap(t[:], ob, odims),
                            _fap(FR[:], (2 * par + dy) * SPAN + ib + dx, idims),
                            bass.AP(tensor=CB[:].tensor,
                                    offset=CB[:].offset + OFF_W + wi * NPTS
                                    + par * N22 + ob,
                                    ap=[list(CB[:].ap[0]),
                                        [2 * N22, n_yi]] + odims[1:]))
                    accs.append(t)
                # accumulate into SMF strided columns (yi = 2m+par)
                dsta = bass.AP(tensor=SMF[:].tensor,
                               offset=SMF[:].offset + par * N22,
                               ap=[list(SMF[:].ap[0]), [2 * N22, n_yi], [1, N22]])
                nc.vector.tensor_add(dsta, _fap(accs[0][:], 0, [[N22, n_yi], [1, N22]]),
                                     _fap(accs[1][:], 0, [[N22, n_yi], [1, N22]]))
                nc.vector.tensor_add(dsta, dsta,
                                     _fap(accs[2][:], 0, [[N22, n_yi], [1, N22]]))
                nc.vector.tensor_add(dsta, dsta,
                                     _fap(accs[3][:], 0, [[N22, n_yi], [1, N22]]))
            tap('SMF', SMF)

            # --- dst points + Hartley (DP/CD in place on SMF) ---
            MN = pool.tile([128, 1], dt)
            nc.vector.scalar_tensor_tensor(
                SMF[:], SMF[:], CBI[:, NPTS:NPTS + 1], CBI[:, 0:NPTS],
                op.mult, op.add)
            tap('DP', SMF)
            nc.vector.reduce_sum(MN[:], SMF[:], axis=mybir.AxisListType.X)
            nc.vector.tensor_scalar_mul(MN[:], MN[:], 1.0 / NPTS)
            nc.vector.tensor_scalar(SMF[:], SMF[:], MN[:], None, op.subtract)

            CDX = pool.tile([64, NPTS], dt)
            CDY = pool.tile([64, NPTS], dt)
            MNP = pool.tile([64, 2], dt)
            cdap = SMF[:]
            pstep = cdap.ap[0][0]
            nc.sync.dma_start(out=CDX[:], in_=bass.AP(
                tensor=cdap.tensor, offset=cdap.offset, ap=[[2 * pstep, 64], [1, NPTS]]))
            nc.sync.dma_start(out=CDY[:], in_=bass.AP(
                tensor=cdap.tensor, offset=cdap.offset + pstep,
                ap=[[2 * pstep, 64], [1, NPTS]]))
            mnap = MN[:]
            nc.sync.dma_start(out=MNP[:], in_=bass.AP(
                tensor=mnap.tensor, offset=mnap.offset, ap=[[mnap.ap[0][0], 128], [1, 1]]))
            tap('CDX', CDX)
            tap('CDY', CDY)
            tap('MNP', MNP)

            R2 = pool.tile([64, NPTS], dt)
            TG = pool.tile([64, NPTS], dt)
            RT = pool.tile([64, NPTS], dt)
            SR = pool.tile([64, 1], dt)
            SC = pool.tile([64, 1], dt)
            IV = pool.tile([64, 1], dt)
            IV2 = pool.tile([64, 1], dt)
            nc.vector.tensor_mul(R2[:], CDX[:], CDX[:])
            nc.vector.tensor_mul(TG[:], CDY[:], CDY[:])
            nc.vector.tensor_add(R2[:], R2[:], TG[:])
            nc.scalar.activation(RT[:], R2[:], act.Sqrt, accum_out=SR[:])
            nc.vector.tensor_scalar(SC[:], SR[:], 1.0 / (NPTS * math.sqrt(2.0)),
                                    1e-8, op.mult, op.max)
            nc.vector.reciprocal(IV[:], SC[:])
            nc.vector.tensor_mul(IV2[:], IV[:], IV[:])
            tap('R2', R2)
            tap('SC', SC)
            tap('IV', IV)
            # normalized dst in place (CDX/CDY -> DNX/DNY)
            nc.vector.tensor_scalar_mul(CDX[:], CDX[:], IV[:])
            nc.vector.tensor_scalar_mul(CDY[:], CDY[:], IV[:])

            # --- g vectors paired on 128 partitions: GU0 = [W2; W2*DNX],
            # GU1 = [W2*DNY; W2*r2n] ---
            GU0 = pool.tile([128, NPTS], dt, tag="t0")
            GU1 = pool.tile([128, NPTS], dt, tag="t1")
            nc.vector.tensor_copy(GU0[0:64, :], W2[:])
            nc.vector.tensor_mul(GU0[64:128, :], W2[:], CDX[:])
            nc.vector.tensor_mul(GU1[0:64, :], W2[:], CDY[:])
            nc.vector.tensor_scalar_mul(TG[:], R2[:], IV2[:])
            nc.vector.tensor_mul(GU1[64:128, :], TG[:], W2[:])

            JK = pool.tile([128, NPTS], dt, tag="t2")
            RD2 = pool.tile([128, 12], dt)
            RD = pool.tile([64, 24], dt)
            for gj, GU in enumerate([GU0, GU1]):
                for k in range(6):
                    nc.vector.scalar_tensor_tensor(
                        JK[:], GU[:], 1.0,
                        CB[:, OFF_C6 + k * NPTS:OFF_C6 + (k + 1) * NPTS],
                        op.bypass, op.mult,
                        accum_out=RD2[:, gj * 6 + k:gj * 6 + k + 1])
            # RD2 partition halves -> RD (64, 24): j order 0,1,2,3
            nc.vector.tensor_copy(_fap(RD[:], 0, [[12, 2], [1, 6]]),
                                  _fap(RD2[0:64], 0, [[6, 2], [1, 6]]))
            nc.vector.tensor_copy(_fap(RD[:], 6, [[12, 2], [1, 6]]),
                                  _fap(RD2[64:128], 0, [[6, 2], [1, 6]]))
            tap('RD', RD)

            # --- assemble M (64, 81) ---
            MM = pool.tile([64, 81], dt)
            EB = pool.tile([64, 4, 9], dt)
            nc.vector.memset(MM[:], 0.0)
            # negate j=1,2 (the -B1/-B2 blocks), then 4 strided copies build
            # all four 9-entry blocks at once: E[j] = RD[j][0,1,2,1,3,4,2,4,5]
            nc.vector.tensor_scalar_mul(RD[:, 6:18], RD[:, 6:18], -1.0)
            nc.vector.tensor_copy(_fap(EB[:], 0, [[9, 4], [1, 3]]),
                                  _fap(RD[:], 0, [[6, 4], [1, 3]]))
            nc.vector.tensor_copy(_fap(EB[:], 3, [[9, 4], [3, 2]]),
                                  _fap(RD[:], 1, [[6, 4], [1, 2]]))
            nc.vector.tensor_copy(_fap(EB[:], 4, [[9, 4], [1, 2]]),
                                  _fap(RD[:], 3, [[6, 4], [1, 2]]))
            nc.vector.tensor_copy(_fap(EB[:], 7, [[9, 4], [1, 2]]),
                                  _fap(RD[:], 4, [[6, 4], [1, 2]]))
            for j, boff in ((0, 0), (0, 30), (3, 60),
                            (1, 6), (1, 54), (2, 33), (2, 57)):
                nc.vector.tensor_copy(
                    _fap(MM[:], boff, [[9, 3], [1, 3]]),
                    _fap(EB[:], j * 9, [[3, 3], [1, 3]]))
            nc.vector.tensor_scalar_add(
                _fap(MM[:], 0, [[10, 9]]), _fap(MM[:], 0, [[10, 9]]), 1e-6)
            if debug:
                MMC = pool.tile([64, 81], dt)
                nc.vector.tensor_copy(MMC[:], MM[:])
                tap('MMpost', MMC)

            # --- LDL^T (in place; strictly-lower cols end up holding -L) ---
            ID = pool.tile([64, 9], dt)
            TMPO = pool.tile([64, 64], dt)
            for k in range(9):
                nc.vector.reciprocal(ID[:, k:k + 1], MM[:, 10 * k:10 * k + 1])
                if k < 8:
                    m = 8 - k
                    col = _fap(MM[:], (k + 1) * 9 + k, [[9, m]])
                    nc.vector.tensor_scalar(col, col, ID[:, k:k + 1], -1.0,
                                            op.mult, op.mult)
                    nc.vector.tensor_mul(
                        _fap(TMPO[:], 0, [[m, m], [1, m]]),
                        _fap(MM[:], (k + 1) * 9 + k, [[9, m], [0, m]]),
                        _fap(MM[:], 9 * k + k + 1, [[0, m], [1, m]]))
                    sub = _fap(MM[:], (k + 1) * 10, [[9, m], [1, m]])
                    nc.vector.tensor_add(sub, sub,
                                         _fap(TMPO[:], 0, [[m, m], [1, m]]))
            tap('ID', ID)

            # --- inverse iteration (2 solves) ---
            Z = pool.tile([64, 9], dt)
            Y = pool.tile([64, 9], dt)
            XN = pool.tile([64, 9], dt)
            TB1 = pool.tile([64, 1], dt)
            for it in range(2):
                if it == 0:
                    nc.vector.tensor_copy(Z[:], CB[0:64, OFF_B0:OFF_B0 + 9])
                    ZT = Z
                else:
                    ZT = XN  # iter-2 forward solve runs in place on XN
                for k in range(8):
                    m = 8 - k
                    nc.vector.scalar_tensor_tensor(
                        ZT[:, k + 1:9],
                        _fap(MM[:], (k + 1) * 9 + k, [[9, m]]),
                        ZT[:, k:k + 1], ZT[:, k + 1:9], op.mult, op.add)
                nc.vector.tensor_mul(Y[:], ZT[:], ID[:])
                nc.vector.tensor_copy(XN[:, 8:9], Y[:, 8:9])
                for k in range(7, -1, -1):
                    m = 8 - k
                    nc.vector.scalar_tensor_tensor(
                        TMPO[:, 0:m],
                        _fap(MM[:], (k + 1) * 9 + k, [[9, m]]),
                        1.0, XN[:, k + 1:9], op.bypass, op.mult,
                        accum_out=TB1[:])
                    nc.vector.tensor_add(XN[:, k:k + 1], Y[:, k:k + 1], TB1[:])
            tap('XN', XN)

            # --- H = T_dst_inv @ (h @ T_src), sign fix, scale ---
            H1 = pool.tile([64, 9], dt)
            TT = pool.tile([64, 9], dt)
            for k in range(3):
                hcol = _fap(XN[:], k, [[3, 3], [0, 3]])
                trow = _fap(CB[0:64], OFF_TS + 3 * k, [[0, 3], [1, 3]])
                if k == 0:
                    nc.vector.tensor_mul(H1[:], hcol, trow)
                else:
                    nc.vector.tensor_mul(TT[:], hcol, trow)
                    nc.vector.tensor_add(H1[:], H1[:], TT[:])
            tap('H1', H1)
            HO = pool.tile([64, 9], dt)
            TB3 = pool.tile([64, 3], dt)
            nc.vector.tensor_scalar_mul(TB3[:], H1[:, 6:9], MNP[:, 0:1])
            nc.vector.scalar_tensor_tensor(HO[:, 0:3], H1[:, 0:3], SC[:], TB3[:],
                                           op.mult, op.add)
            nc.vector.tensor_scalar_mul(TB3[:], H1[:, 6:9], MNP[:, 1:2])
            nc.vector.scalar_tensor_tensor(HO[:, 3:6], H1[:, 3:6], SC[:], TB3[:],
                                           op.mult, op.add)
            nc.vector.tensor_copy(HO[:, 6:9], H1[:, 6:9])

            FF = pool.tile([64, 1], dt)
            nc.vector.reciprocal(FF[:], HO[:, 8:9])
            nc.vector.tensor_scalar_mul(HO[:], HO[:], FF[:])

            nc.sync.dma_start(out=hout_d[:], in_=HO[:])

    _split_multi_waits(nc)
    nc._dbg_taps = taps
    return nc


_PROG = {}


def _get_prog(img_h, img_w):
    key = (int(img_h), int(img_w))
    if key not in _PROG:
        cb, cbi, rows = _constants(*key)
        nc = _build_program(rows)
        _PROG[key] = (nc, cb, cbi)
    return _PROG[key]


def _make_in_maps(flow, mask, cb, cbi):
    in_maps = []
    for c in range(NCORES):
        sl = slice(c * BPC, (c + 1) * BPC)
        in_maps.append({
            "flow": np.ascontiguousarray(flow[sl]),
            "mask": np.ascontiguousarray(mask[sl]),
            "cb": cb,
            "cbi": cbi,
        })
    return in_maps


def run_spmd(flow, mask, img_h, img_w, **kw):
    """Compile (cached) + run on 8 cores. Returns BassKernelResults."""
    from concourse.bass_utils import run_bass_kernel_spmd
    nc, cb, cbi = _get_prog(img_h, img_w)
    in_maps = _make_in_maps(flow, mask, cb, cbi)
    return run_bass_kernel_spmd(nc, in_maps, core_ids=list(range(NCORES)), **kw)


def kernel(flow, mask, img_h, img_w):
    flow = np.ascontiguousarray(np.asarray(flow), dtype=f32)
    mask = np.ascontiguousarray(np.asarray(mask), dtype=f32)
    res = run_spmd(flow, mask, int(img_h), int(img_w))
    out = np.concatenate([r["hout"] for r in res.results], axis=0)
    return out.reshape(flow.shape[0], 3, 3).astype(f32)
